# revision 15
# baseline (speedup 1.0000x reference)
"""Trainium2 Bass kernel for nn_DECSeq3 (DynamicEdgeConv over streamlines).

Self-contained: hardcodes shapes from the problem spec.
  pos [131072, 3] f32, edge_index [2, 245760] int64, plus MLP weights.
  Output [8192, 2] f32.

Strategy: data-parallel over the 8192 streamlines across 8 NeuronCores
(1024 streamlines/core).  BatchNorm affines folded into downstream
weights on the host.  Per core, per tile of 1024 padded nodes:
  - stage1 pointwise convs with channel-duplicated weights [W|W] so both
    x and its square land partition-aligned (no shift DMA)
  - per-streamline kNN: one [128,128] f32 distance matmul per node block
    (psi/phi trick), additive block-diag mask, full-row max8/max_index
  - neighbor gather WITHOUT indirect DMA: per-k one-hot built by a
    gpsimd tensor_scalar is_equal against an iota constant, transposed
    on PE, then gathered as fp16 matmuls G_k = Bn^T @ SelT_k
  - edge MLP via A/B decomposition: x2 = relu(A[p] + max_k B[nbr_k(p)])
  - l1 matmul (fp16) + max-pool over points, then m1/m2/m3 head (f32r).
"""

import os
import sys

if "/opt/trn_rl_repo" not in sys.path:
    sys.path.insert(0, "/opt/trn_rl_repo")

import numpy as np

# ---------------- problem constants ----------------
B_FULL = 8192
L = 16
D = 3
K = 5
NCLS = 2
P = L - 1          # 15 real points per streamline
PP = 16            # padded points
EPS = 1e-5

NCORES = 8
BC = 1024          # streamlines per core
NODES = BC * PP    # 16384 padded nodes per core
NTILES = 16
TNODES = NODES // NTILES      # 1024 nodes per tile
TSTRL = BC // NTILES          # 64 streamlines per tile
NBLK = TNODES // 128          # 8 blocks of 128 nodes per tile
BIG_NEG = -1.0e30

_CACHE = {}


# ---------------- device program ----------------
def _build_program():
    import concourse.bacc as bacc
    import concourse.bass as bass
    import concourse.mybir as mybir
    from concourse.tile import TileContext
    from concourse.masks import make_identity

    dt = mybir.dt
    f32 = dt.float32
    f32r = dt.float32r
    f16 = dt.float16
    u16 = dt.uint16
    AF = mybir.ActivationFunctionType
    OP = mybir.AluOpType
    AX = mybir.AxisListType

    nc = bacc.Bacc("TRN2", target_bir_lowering=False)

    # ---- DRAM I/O ----
    xefw = nc.dram_tensor("xefw", [7, NODES], f32, kind="ExternalInput")
    xebw = nc.dram_tensor("xebw", [7, NODES], f32, kind="ExternalInput")
    s1wf = nc.dram_tensor("s1wf", [7, 128], f32, kind="ExternalInput")
    s1wb = nc.dram_tensor("s1wb", [7, 128], f32, kind="ExternalInput")
    s1g = nc.dram_tensor("s1g", [128, 1], f32, kind="ExternalInput")
    s1b = nc.dram_tensor("s1b", [128, 1], f32, kind="ExternalInput")
    wa = nc.dram_tensor("wa", [65, 128], f16, kind="ExternalInput")
    wdt = nc.dram_tensor("wdt", [64, 128], f16, kind="ExternalInput")
    wl1x1 = nc.dram_tensor("wl1x1", [65, 1024], f16, kind="ExternalInput")
    wl1x2 = nc.dram_tensor("wl1x2", [128, 1024], f16, kind="ExternalInput")
    # m-layer weights pre-arranged on host to [128, kchunks*M]
    wm1 = nc.dram_tensor("wm1", [128, 8 * 512], f32r, kind="ExternalInput")
    bm1 = nc.dram_tensor("bm1", [1, 512], f32r, kind="ExternalInput")
    wm2 = nc.dram_tensor("wm2", [128, 4 * 256], f32r, kind="ExternalInput")
    bm2 = nc.dram_tensor("bm2", [1, 256], f32r, kind="ExternalInput")
    wm3 = nc.dram_tensor("wm3", [128, 2 * 2], f32r, kind="ExternalInput")
    bm3 = nc.dram_tensor("bm3", [1, 2], f32r, kind="ExternalInput")
    maskbd = nc.dram_tensor("maskbd", [128, 128], f32, kind="ExternalInput")
    iotaq = nc.dram_tensor("iotaq", [128, 128], f16, kind="ExternalInput")
    onesr = nc.dram_tensor("onesr", [1, BC], f32r, kind="ExternalInput")
    out_t = nc.dram_tensor("out", [2, BC], f32, kind="ExternalOutput")

    ABLS = set(os.environ.get("KABL", "").split(","))

    with TileContext(nc) as tc:
        with tc.tile_pool(name="const", bufs=1) as cpool, \
             tc.tile_pool(name="wpool", bufs=1) as wpool, \
             tc.tile_pool(name="pooled", bufs=1) as plpool, \
             tc.tile_pool(name="head", bufs=1) as headp:

            ident16 = cpool.tile([128, 128], f16)
            make_identity(nc, ident16[:])
            ones_row = cpool.tile([1, BC], f32r)
            nc.scalar.dma_start(out=ones_row[:], in_=onesr[:])
            t_mask = cpool.tile([128, 128], f32)
            nc.scalar.dma_start(out=t_mask[:], in_=maskbd[:])
            t_iota = cpool.tile([128, 128], f16)
            nc.scalar.dma_start(out=t_iota[:], in_=iotaq[:])

            t_s1wf = wpool.tile([7, 128], f32)
            nc.scalar.dma_start(out=t_s1wf[:], in_=s1wf[:])
            t_s1wb = wpool.tile([7, 128], f32)
            nc.scalar.dma_start(out=t_s1wb[:], in_=s1wb[:])
            t_s1g = wpool.tile([128, 1], f32)
            nc.scalar.dma_start(out=t_s1g[:], in_=s1g[:])
            t_s1b = wpool.tile([128, 1], f32)
            nc.scalar.dma_start(out=t_s1b[:], in_=s1b[:])
            t_wa = wpool.tile([65, 128], f16)
            nc.scalar.dma_start(out=t_wa[:], in_=wa[:])
            t_wdt = wpool.tile([64, 128], f16)
            nc.scalar.dma_start(out=t_wdt[:], in_=wdt[:])
            t_wl1x1 = wpool.tile([65, 1024], f16)
            nc.scalar.dma_start(out=t_wl1x1[:], in_=wl1x1[:])
            t_wl1x2 = wpool.tile([128, 1024], f16)
            nc.scalar.dma_start(out=t_wl1x2[:], in_=wl1x2[:])
            t_wm1 = wpool.tile([128, 8 * 512], f32r)
            nc.scalar.dma_start(out=t_wm1[:], in_=wm1[:])
            t_bm1 = wpool.tile([1, 512], f32r)
            nc.scalar.dma_start(out=t_bm1[:], in_=bm1[:])
            t_wm2 = wpool.tile([128, 4 * 256], f32r)
            nc.scalar.dma_start(out=t_wm2[:], in_=wm2[:])
            t_bm2 = wpool.tile([1, 256], f32r)
            nc.scalar.dma_start(out=t_bm2[:], in_=bm2[:])
            t_wm3 = wpool.tile([128, 4], f32r)
            nc.scalar.dma_start(out=t_wm3[:], in_=wm3[:])
            t_bm3 = wpool.tile([1, 2], f32r)
            nc.scalar.dma_start(out=t_bm3[:], in_=bm3[:])

            # pooled pre-activations, one [128, BC] buffer per 128-ch chunk
            pooled = [plpool.tile([128, BC], f32r, name=f"pooled{m}",
                                  tag=f"pooled{m}") for m in range(8)]

            with tc.tile_pool(name="io", bufs=2) as iop, \
                 tc.tile_pool(name="s1st", bufs=2) as s1st, \
                 tc.tile_pool(name="xt", bufs=2) as xtp, \
                 tc.tile_pool(name="knn", bufs=4) as knnp, \
                 tc.tile_pool(name="ps_a", bufs=3, space="PSUM") as ps_a, \
                 tc.tile_pool(name="ps_sel", bufs=1, space="PSUM") as ps_sel, \
                 tc.tile_pool(name="ps_g", bufs=1, space="PSUM") as ps_g, \
                 tc.tile_pool(name="ps_big", bufs=2, space="PSUM") as ps_big:

                def l1_unit(t, m, x1t, x2t):
                    if "nol1" in ABLS:
                        return
                    # l1 matmuls + max-pool for one 128-ch chunk of tile t.
                    # Both halves share each lhsT so bass can skip reloads.
                    pls = []
                    for h in range(2):
                        pl = ps_big.tile([128, 512], f32, tag="big",
                                         name=f"pl{h}")
                        nc.tensor.matmul(
                            out=pl[:],
                            lhsT=t_wl1x1[:, m * 128:(m + 1) * 128],
                            rhs=x1t[:, h * 512:(h + 1) * 512],
                            start=True, stop=False)
                        pls.append(pl)
                    for h in range(2):
                        nc.tensor.matmul(
                            out=pls[h][:],
                            lhsT=t_wl1x2[:, m * 128:(m + 1) * 128],
                            rhs=x2t[:, h * 512:(h + 1) * 512],
                            start=False, stop=True)
                    for h in range(2):
                        pv = pls[h][:].rearrange("p (s q) -> p s q",
                                                 q=16)[:, :, 0:15]
                        psl = slice(t * TSTRL + h * 32,
                                    t * TSTRL + (h + 1) * 32)
                        nc.vector.tensor_reduce(out=pooled[m][:, psl],
                                                in_=pv, axis=AX.X, op=OP.max)

                def load_tile(t):
                    c0 = t * TNODES
                    xfc = iop.tile([7, TNODES], f32, tag="xfc")
                    xbc = iop.tile([7, TNODES], f32, tag="xbc")
                    nc.sync.dma_start(out=xfc[:], in_=xefw[:, c0:c0 + TNODES])
                    nc.sync.dma_start(out=xbc[:], in_=xebw[:, c0:c0 + TNODES])
                    return xfc, xbc

                def do_tile(t, prev, cur_io, nxt_io):
                    xfc, xbc = cur_io

                    x1g = xtp.tile([128, TNODES], f32, tag="x1g")
                    x1r2 = xtp.tile([128, TNODES], f32, tag="x1r2")
                    x1t = xtp.tile([65, TNODES], f16, tag="x1t")
                    x2t = xtp.tile([128, TNODES], f16, tag="x2t")
                    nc.vector.memset(x1g[64:128, :], -1.0)
                    nc.vector.memset(x1t[64:65, :], -1.0)

                    for ch in range(2):
                        sl = slice(ch * 512, (ch + 1) * 512)
                        pf = ps_big.tile([128, 512], f32, tag="big")
                        nc.tensor.matmul(out=pf[:], lhsT=t_s1wf[:],
                                         rhs=xfc[:, sl], start=True, stop=True)
                        fwa = s1st.tile([128, 512], f32, tag="fwa")
                        nc.scalar.activation(out=fwa[:], in_=pf[:], func=AF.Relu,
                                             bias=t_s1b[:], scale=t_s1g[:])
                        pb = ps_big.tile([128, 512], f32, tag="big")
                        nc.tensor.matmul(out=pb[:], lhsT=t_s1wb[:],
                                         rhs=xbc[:, sl], start=True, stop=True)
                        bwa = s1st.tile([128, 512], f32, tag="bwa")
                        nc.scalar.activation(out=bwa[:], in_=pb[:], func=AF.Relu,
                                             bias=t_s1b[:], scale=t_s1g[:])
                        xd = s1st.tile([128, 512], f32, tag="xd")
                        nc.vector.tensor_tensor(out=xd[:], in0=fwa[:],
                                                in1=bwa[:], op=OP.add)
                        nc.scalar.copy(out=x1g[0:64, sl], in_=xd[0:64, :])
                        nc.scalar.copy(out=x1t[0:64, sl], in_=xd[0:64, :])
                        nc.scalar.activation(out=x1r2[0:64, sl], in_=xd[0:64, :],
                                             func=AF.Copy, scale=2.0)
                        nc.scalar.activation(out=x1r2[64:128, sl],
                                             in_=xd[64:128, :], func=AF.Square)

                    # ---- kNN + gather + edge MLP, software-pipelined; the
                    # previous tile's l1 units are interleaved to keep every
                    # engine fed while per-block chains cross engines ----
                    SKIP = "noknn" in ABLS
                    Hs, SelTs, Bns, pas = {}, {}, {}, {}
                    for b in range(NBLK + 3 if not SKIP else 0):
                        if b < NBLK:
                            bs = slice(b * 128, (b + 1) * 128)
                            pd = ps_a.tile([128, 128], f32, tag="a")
                            nc.tensor.matmul(out=pd[:], lhsT=x1g[:, bs],
                                             rhs=x1r2[:, bs], start=True,
                                             stop=True)
                            if "nosel" not in ABLS:
                                pdm = knnp.tile([128, 128], f32, tag="pdm")
                                if "nomask" not in ABLS:
                                    nc.vector.tensor_tensor(
                                        out=pdm[:], in0=pd[:],
                                        in1=t_mask[:], op=OP.add)
                                else:
                                    nc.vector.memset(pdm[:], 1.0)
                                m8 = knnp.tile([128, 8], f32, tag="m8")
                                i8 = knnp.tile([128, 8], u16, tag="i8")
                                if "nomax8" not in ABLS:
                                    nc.vector.max(out=m8[:], in_=pdm[:])
                                    nc.vector.max_index(out=i8[:], in_max=m8[:],
                                                        in_values=pdm[:])
                                else:
                                    nc.vector.memset(m8[:], 1.0)
                                    nc.vector.memset(i8[:], 3)
                                i8f = knnp.tile([128, 8], f16, tag="i8f")
                                nc.vector.tensor_copy(out=i8f[:], in_=i8[:])
                                # one-hots for neighbors k=1..4 (k=0 is self,
                                # gathered for free via the identity matrix);
                                # one DVE compare for all four k
                                H = knnp.tile([128, 4 * 128], f16, tag="H")
                                if "noiseq" not in ABLS:
                                    nc.vector.tensor_tensor(
                                        out=H[:].rearrange(
                                            "p (k q) -> p k q", k=4),
                                        in0=t_iota[:].unsqueeze(1)
                                        .to_broadcast([128, 4, 128]),
                                        in1=i8f[:, 1:5].unsqueeze(2)
                                        .to_broadcast([128, 4, 128]),
                                        op=OP.is_equal)
                                else:
                                    nc.vector.memset(H[:], 0.0)
                                Hs[b] = H
                            # B features (node-major) and A term
                            pbm = ps_a.tile([128, 128], f32, tag="a")
                            nc.tensor.matmul(out=pbm[:], lhsT=x1t[0:64, bs],
                                             rhs=t_wdt[:], start=True, stop=True)
                            Bn = knnp.tile([128, 128], f16, tag="Bn")
                            nc.scalar.copy(out=Bn[:], in_=pbm[:])
                            Bns[b] = Bn
                            pa = ps_a.tile([128, 128], f32, tag="a")
                            nc.tensor.matmul(out=pa[:], lhsT=t_wa[:],
                                             rhs=x1t[:, bs], start=True,
                                             stop=True)
                            An = knnp.tile([128, 128], f32, tag="An")
                            nc.scalar.copy(out=An[:], in_=pa[:])
                            pas[b] = An
                        if prev is not None and b < 8:
                            l1_unit(t - 1, b, prev["x1t"], prev["x2t"])
                        if 1 <= b < NBLK + 1 and "nosel" not in ABLS:
                            # transpose previous block's one-hots
                            H = Hs.pop(b - 1)
                            selp = ps_sel.tile([128, 4 * 128], f16, tag="sel")
                            if "notp" not in ABLS:
                                for k in range(4):
                                    nc.tensor.transpose(
                                        out=selp[:, k * 128:(k + 1) * 128],
                                        in_=H[:, k * 128:(k + 1) * 128],
                                        identity=ident16[:])
                            else:
                                nc.vector.memset(selp[:], 0.0)
                            SelT = knnp.tile([128, 4 * 128], f16, tag="SelT")
                            if "noselcp" not in ABLS:
                                nc.scalar.copy(out=SelT[:], in_=selp[:])
                            else:
                                nc.vector.memset(SelT[:], 0.0)
                            SelTs[b - 1] = SelT
                        if b >= 3 and "nogather" not in ABLS:
                            bb = b - 3
                            bs = slice(bb * 128, (bb + 1) * 128)
                            SelT = (SelTs.pop(bb) if "nosel" not in ABLS
                                    else None)
                            Bn = Bns.pop(bb)
                            An = pas.pop(bb)
                            G = ps_g.tile([128, 5 * 128], f32, tag="g")
                            nc.tensor.matmul(out=G[:, 0:512], lhsT=Bn[:],
                                             rhs=(SelT[:] if SelT is not None
                                                  else t_wl1x2[:, 0:512]),
                                             start=True, stop=True)
                            nc.tensor.matmul(out=G[:, 512:640], lhsT=Bn[:],
                                             rhs=ident16[:], start=True,
                                             stop=True)
                            x2m = knnp.tile([128, 128], f32, tag="x2m")
                            nc.vector.tensor_reduce(
                                out=x2m[:],
                                in_=G[:].rearrange("c (k p) -> c p k", k=K),
                                axis=AX.X, op=OP.max)
                            x2n = knnp.tile([128, 128], f32, tag="x2n")
                            nc.vector.tensor_tensor(out=x2n[:], in0=x2m[:],
                                                    in1=An[:], op=OP.add)
                            nc.scalar.activation(out=x2t[:, bs], in_=x2n[:],
                                                 func=AF.Relu)
                    if SKIP:
                        nc.vector.memset(x2t[:], 0.0)
                        if prev is not None:
                            for m in range(8):
                                l1_unit(t - 1, m, prev["x1t"], prev["x2t"])
                    return {"x1t": x1t, "x2t": x2t}

                t1 = [headp.tile([128, BC], f32r, name=f"t1_{o}",
                                 tag=f"t1_{o}") for o in range(4)]
                t2 = [headp.tile([128, BC], f32r, name=f"t2_{o}",
                                 tag=f"t2_{o}") for o in range(2)]
                outs = headp.tile([2, BC], f32, tag="outs")

                def head_half(h):
                    if "nohead" in ABLS:
                        return
                    # ---- head: relu, m1, m2, m3 for one half of the batch;
                    # half 0 only needs tiles 0-7 pooled, so it can run while
                    # tiles 8-15 are still in flight ----
                    osl = slice(h * 512, (h + 1) * 512)
                    h1 = pooled
                    for m in range(8):
                        nc.scalar.activation(out=h1[m][:, osl],
                                             in_=h1[m][:, osl], func=AF.Relu)
                    wm1v = t_wm1[:].rearrange("p (a m) -> p a m", a=8)
                    for o in range(4):
                        pm1 = ps_big.tile([128, 512], f32, tag="big")
                        for kc in range(8):
                            nc.tensor.matmul(
                                out=pm1[:],
                                lhsT=wm1v[:, kc, o * 128:(o + 1) * 128],
                                rhs=h1[kc][:, osl],
                                start=(kc == 0), stop=False)
                        nc.tensor.matmul(
                            out=pm1[:],
                            lhsT=t_bm1[:, o * 128:(o + 1) * 128],
                            rhs=ones_row[:, osl],
                            start=False, stop=True)
                        nc.scalar.activation(out=t1[o][:, osl], in_=pm1[:],
                                             func=AF.Relu)
                    wm2v = t_wm2[:].rearrange("p (a m) -> p a m", a=4)
                    for o in range(2):
                        pm2 = ps_big.tile([128, 512], f32, tag="big")
                        for kc in range(4):
                            nc.tensor.matmul(
                                out=pm2[:],
                                lhsT=wm2v[:, kc, o * 128:(o + 1) * 128],
                                rhs=t1[kc][:, osl],
                                start=(kc == 0), stop=False)
                        nc.tensor.matmul(
                            out=pm2[:],
                            lhsT=t_bm2[:, o * 128:(o + 1) * 128],
                            rhs=ones_row[:, osl],
                            start=False, stop=True)
                        nc.scalar.activation(out=t2[o][:, osl], in_=pm2[:],
                                             func=AF.Relu)
                    wm3v = t_wm3[:].rearrange("p (a m) -> p a m", a=2)
                    pm3 = ps_big.tile([128, 512], f32, tag="big")
                    for kc in range(2):
                        nc.tensor.matmul(
                            out=pm3[0:2, :],
                            lhsT=wm3v[:, kc, :],
                            rhs=t2[kc][:, osl],
                            start=(kc == 0), stop=False)
                    nc.tensor.matmul(out=pm3[0:2, :],
                                     lhsT=t_bm3[:],
                                     rhs=ones_row[:, osl],
                                     start=False, stop=True)
                    nc.scalar.copy(out=outs[:, osl], in_=pm3[0:2, :])

                def whole_body():
                    prev = None
                    cur_io = load_tile(0)
                    for t in range(NTILES):
                        nxt_io = load_tile(t + 1) if t + 1 < NTILES else None
                        prev = do_tile(t, prev, cur_io, nxt_io)
                        cur_io = nxt_io
                        if t == 10:
                            head_half(0)
                    for m in range(8):
                        l1_unit(NTILES - 1, m, prev["x1t"], prev["x2t"])
                    head_half(1)
                    nc.sync.dma_start(out=out_t[:], in_=outs[:])

                REPEAT = int(os.environ.get("KREPEAT", "1"))
                if REPEAT > 1:
                    with tc.For_i(0, REPEAT, 1):
                        whole_body()
                else:
                    whole_body()

    nc.finalize()
    return nc


# ---------------- host-side prep ----------------
def _prep_inputs(pos, edge_index,
                 W_c1fw, b_c1fw, W_c1bw, b_c1bw, g_bn1, be_bn1,
                 W_e, b_e, g_e, be_e,
                 W_l1, b_l1, g_l1, be_l1,
                 W_m1, b_m1, g_m1, be_m1,
                 W_m2, b_m2, g_m2, be_m2,
                 W_m3, b_m3):
    f = np.float32
    h = np.float16
    pos = np.asarray(pos, f)
    E = edge_index.shape[1]
    N = E // 2
    second = np.asarray(edge_index[:, N:])
    first = second[:, ::-1]
    src = np.concatenate([first[0], second[0]])
    dst = np.concatenate([first[1], second[1]])
    xe = np.concatenate([pos[dst] - pos[src], pos[src]], axis=1).astype(f)
    xe = xe.reshape(2 * B_FULL, P, 2 * D)
    fw = xe[:B_FULL]
    bw = xe[B_FULL:][::-1, ::-1, :]

    def pad_t(a):
        # [B, 15, 6] -> per-core feature-major [7, NODES] with ones row
        out = np.zeros((B_FULL, PP, 7), f)
        out[:, :P, :6] = a
        out[:, :, 6] = 1.0
        out = out.reshape(NCORES, NODES, 7)
        return np.ascontiguousarray(out.transpose(0, 2, 1))

    xefw = pad_t(fw)
    xebw = pad_t(bw)

    sq = np.sqrt(np.asarray(1.0 + EPS, f))
    g1 = (np.asarray(g_bn1, f) / sq)
    be1 = np.asarray(be_bn1, f)
    s1g = np.ascontiguousarray(np.tile(g1, 2)[:, None])
    s1b = np.ascontiguousarray(np.tile(be1, 2)[:, None])
    s1wf = np.ascontiguousarray(np.tile(
        np.concatenate([np.asarray(W_c1fw, f),
                        np.asarray(b_c1fw, f)[:, None]], 1).T, (1, 2)))
    s1wb = np.ascontiguousarray(np.tile(
        np.concatenate([np.asarray(W_c1bw, f),
                        np.asarray(b_c1bw, f)[:, None]], 1).T, (1, 2)))

    W_e = np.asarray(W_e, f)
    Wi, Wd = W_e[:, :64], W_e[:, 64:]
    wa = np.ascontiguousarray(
        np.concatenate([(Wi - Wd).T, -np.asarray(b_e, f)[None, :]], 0)).astype(h)
    wdt = np.ascontiguousarray(Wd.T).astype(h)

    ge = np.asarray(g_e, f) / sq
    bee = np.asarray(be_e, f)
    W_l1 = np.asarray(W_l1, f)
    Wl1x1 = W_l1[:, :64]
    Wl1x2 = W_l1[:, 64:] * ge[None, :]
    bl1 = np.asarray(b_l1, f) + W_l1[:, 64:] @ bee
    wl1x1 = np.ascontiguousarray(
        np.concatenate([Wl1x1.T, -bl1[None, :]], 0)).astype(h)
    wl1x2 = np.ascontiguousarray(Wl1x2.T).astype(h)

    def m_fold(W, b, g_prev, be_prev, kchunks):
        # fold previous-layer bn affine into this layer; arrange lhsT
        # [K, M] -> [128, kchunks*M]
        W = np.asarray(W, f)
        gp = np.asarray(g_prev, f) / sq
        Wf = W * gp[None, :]
        bf = np.asarray(b, f) + W @ np.asarray(be_prev, f)
        lhsT = Wf.T  # [K, M]
        Kd, Md = lhsT.shape
        arr = lhsT.reshape(kchunks, 128, Md).transpose(1, 0, 2).reshape(128, -1)
        return np.ascontiguousarray(arr), bf[None, :]

    wm1a, bm1v = m_fold(W_m1, b_m1, g_l1, be_l1, 8)
    wm2a, bm2v = m_fold(W_m2, b_m2, g_m1, be_m1, 4)
    wm3a, bm3v = m_fold(W_m3, b_m3, g_m2, be_m2, 2)

    # additive block-diagonal mask: 0 on own-streamline non-pad columns
    pi = np.arange(128)
    qi = np.arange(128)
    ok = ((pi[:, None] // 16) == (qi[None, :] // 16)) & ((qi[None, :] % 16) != 15)
    maskbd = np.where(ok, 0.0, BIG_NEG).astype(f)
    iotaq = np.tile(np.arange(128, dtype=h)[None, :], (128, 1))

    shared = {
        "s1wf": s1wf, "s1wb": s1wb, "s1g": s1g, "s1b": s1b,
        "wa": wa, "wdt": wdt,
        "wl1x1": wl1x1, "wl1x2": wl1x2,
        "wm1": wm1a, "bm1": bm1v,
        "wm2": wm2a, "bm2": bm2v,
        "wm3": wm3a, "bm3": bm3v,
        "maskbd": maskbd,
        "iotaq": np.ascontiguousarray(iotaq),
        "onesr": np.ones((1, BC), f),
    }
    in_maps = []
    for c in range(NCORES):
        m = dict(shared)
        m["xefw"] = xefw[c]
        m["xebw"] = xebw[c]
        in_maps.append(m)
    return in_maps


def _get_runner():
    if "runner" in _CACHE:
        return _CACHE["runner"]
    from concourse.bass_utils import run_bass_kernel_spmd
    nc = _build_program()
    _CACHE["nc"] = nc

    def runner(in_maps):
        return run_bass_kernel_spmd(nc, in_maps, list(range(NCORES))).results

    _CACHE["runner"] = runner
    return runner


def kernel(**inputs):
    in_maps = _prep_inputs(**inputs)
    results = _get_runner()(in_maps)
    out = np.empty((B_FULL, NCLS), np.float32)
    for c in range(NCORES):
        out[c * BC:(c + 1) * BC, :] = results[c]["out"].T
    return out


# revision 17
# speedup vs baseline: 1.3052x; 1.3052x over previous
"""Trainium2 Bass kernel for nn_DECSeq3 (DynamicEdgeConv over streamlines).

Self-contained: hardcodes shapes from the problem spec.
  pos [131072, 3] f32, edge_index [2, 245760] int64, plus MLP weights.
  Output [8192, 2] f32.

Strategy: data-parallel over the 8192 streamlines across 8 NeuronCores
(1024 streamlines/core).  BatchNorm affines folded into downstream
weights on the host.  Per core, per tile of 1024 padded nodes:
  - stage1 pointwise convs with channel-duplicated weights [W|W] so both
    x and its square land partition-aligned (no shift DMA)
  - per-streamline kNN: one [128,128] f32 distance matmul per node block
    (psi/phi trick), additive block-diag mask, full-row max8/max_index
  - neighbor gather WITHOUT indirect DMA: per-k one-hot built by a
    gpsimd tensor_scalar is_equal against an iota constant, transposed
    on PE, then gathered as fp16 matmuls G_k = Bn^T @ SelT_k
  - edge MLP via A/B decomposition: x2 = relu(A[p] + max_k B[nbr_k(p)])
  - l1 matmul (fp16) + max-pool over points, then m1/m2/m3 head (f32r).
"""

import os
import sys

if "/opt/trn_rl_repo" not in sys.path:
    sys.path.insert(0, "/opt/trn_rl_repo")

import numpy as np

# ---------------- problem constants ----------------
B_FULL = 8192
L = 16
D = 3
K = 5
NCLS = 2
P = L - 1          # 15 real points per streamline
PP = 16            # padded points
EPS = 1e-5

NCORES = 8
BC = 1024          # streamlines per core
NODES = BC * PP    # 16384 padded nodes per core
NTILES = 16
TNODES = NODES // NTILES      # 1024 nodes per tile
TSTRL = BC // NTILES          # 64 streamlines per tile
NBLK = TNODES // 128          # 8 blocks of 128 nodes per tile
BIG_NEG = -1.0e30

_CACHE = {}


# ---------------- device program ----------------
def _build_program():
    import concourse.bacc as bacc
    import concourse.bass as bass
    import concourse.mybir as mybir
    from concourse.tile import TileContext
    from concourse.masks import make_identity

    dt = mybir.dt
    f32 = dt.float32
    f32r = dt.float32r
    f16 = dt.float16
    u16 = dt.uint16
    AF = mybir.ActivationFunctionType
    OP = mybir.AluOpType
    AX = mybir.AxisListType

    nc = bacc.Bacc("TRN2", target_bir_lowering=False)

    # ---- DRAM I/O ----
    xefw = nc.dram_tensor("xefw", [7, NODES], f32, kind="ExternalInput")
    xebw = nc.dram_tensor("xebw", [7, NODES], f32, kind="ExternalInput")
    s1wf = nc.dram_tensor("s1wf", [7, 128], f32, kind="ExternalInput")
    s1wb = nc.dram_tensor("s1wb", [7, 128], f32, kind="ExternalInput")
    s1g = nc.dram_tensor("s1g", [128, 1], f32, kind="ExternalInput")
    s1b = nc.dram_tensor("s1b", [128, 1], f32, kind="ExternalInput")
    wa = nc.dram_tensor("wa", [65, 128], f16, kind="ExternalInput")
    wdt = nc.dram_tensor("wdt", [64, 128], f16, kind="ExternalInput")
    wl1x1 = nc.dram_tensor("wl1x1", [65, 1024], f16, kind="ExternalInput")
    wl1x2 = nc.dram_tensor("wl1x2", [128, 1024], f16, kind="ExternalInput")
    # m-layer weights pre-arranged on host to [128, kchunks*M]
    wm1 = nc.dram_tensor("wm1", [128, 8 * 512], f32r, kind="ExternalInput")
    bm1 = nc.dram_tensor("bm1", [1, 512], f32r, kind="ExternalInput")
    wm2 = nc.dram_tensor("wm2", [128, 4 * 256], f32r, kind="ExternalInput")
    bm2 = nc.dram_tensor("bm2", [1, 256], f32r, kind="ExternalInput")
    wm3 = nc.dram_tensor("wm3", [128, 2 * 2], f32r, kind="ExternalInput")
    bm3 = nc.dram_tensor("bm3", [1, 2], f32r, kind="ExternalInput")
    maskbd = nc.dram_tensor("maskbd", [128, 128], f32, kind="ExternalInput")
    iotaq = nc.dram_tensor("iotaq", [128, 128], f16, kind="ExternalInput")
    onesr = nc.dram_tensor("onesr", [1, BC], f32r, kind="ExternalInput")
    out_t = nc.dram_tensor("out", [2, BC], f32, kind="ExternalOutput")

    ABLS = set(os.environ.get("KABL", "").split(","))

    with TileContext(nc) as tc:
        with tc.tile_pool(name="const", bufs=1) as cpool, \
             tc.tile_pool(name="wpool", bufs=1) as wpool, \
             tc.tile_pool(name="pooled", bufs=1) as plpool, \
             tc.tile_pool(name="head", bufs=1) as headp:

            ident16 = cpool.tile([128, 128], f16)
            make_identity(nc, ident16[:])
            ones_row = cpool.tile([1, BC], f32r)
            nc.scalar.dma_start(out=ones_row[:], in_=onesr[:])
            t_mask = cpool.tile([128, 128], f32)
            nc.scalar.dma_start(out=t_mask[:], in_=maskbd[:])
            t_iota = cpool.tile([128, 128], f16)
            nc.scalar.dma_start(out=t_iota[:], in_=iotaq[:])

            t_s1wf = wpool.tile([7, 128], f32)
            nc.scalar.dma_start(out=t_s1wf[:], in_=s1wf[:])
            t_s1wb = wpool.tile([7, 128], f32)
            nc.scalar.dma_start(out=t_s1wb[:], in_=s1wb[:])
            t_s1g = wpool.tile([128, 1], f32)
            nc.scalar.dma_start(out=t_s1g[:], in_=s1g[:])
            t_s1b = wpool.tile([128, 1], f32)
            nc.scalar.dma_start(out=t_s1b[:], in_=s1b[:])
            t_wa = wpool.tile([65, 128], f16)
            nc.scalar.dma_start(out=t_wa[:], in_=wa[:])
            t_wdt = wpool.tile([64, 128], f16)
            nc.scalar.dma_start(out=t_wdt[:], in_=wdt[:])
            t_wl1x1 = wpool.tile([65, 1024], f16)
            nc.scalar.dma_start(out=t_wl1x1[:], in_=wl1x1[:])
            t_wl1x2 = wpool.tile([128, 1024], f16)
            nc.scalar.dma_start(out=t_wl1x2[:], in_=wl1x2[:])
            t_wm1 = wpool.tile([128, 8 * 512], f32r)
            nc.scalar.dma_start(out=t_wm1[:], in_=wm1[:])
            t_bm1 = wpool.tile([1, 512], f32r)
            nc.scalar.dma_start(out=t_bm1[:], in_=bm1[:])
            t_wm2 = wpool.tile([128, 4 * 256], f32r)
            nc.scalar.dma_start(out=t_wm2[:], in_=wm2[:])
            t_bm2 = wpool.tile([1, 256], f32r)
            nc.scalar.dma_start(out=t_bm2[:], in_=bm2[:])
            t_wm3 = wpool.tile([128, 4], f32r)
            nc.scalar.dma_start(out=t_wm3[:], in_=wm3[:])
            t_bm3 = wpool.tile([1, 2], f32r)
            nc.scalar.dma_start(out=t_bm3[:], in_=bm3[:])

            # pooled pre-activations, one [128, BC] buffer per 128-ch chunk
            pooled = [plpool.tile([128, BC], f32r, name=f"pooled{m}",
                                  tag=f"pooled{m}") for m in range(8)]

            with tc.tile_pool(name="io", bufs=2) as iop, \
                 tc.tile_pool(name="s1st", bufs=2) as s1st, \
                 tc.tile_pool(name="xt", bufs=2) as xtp, \
                 tc.tile_pool(name="knn", bufs=4) as knnp, \
                 tc.tile_pool(name="ps_a", bufs=2, space="PSUM") as ps_a, \
                 tc.tile_pool(name="ps_sel", bufs=1, space="PSUM") as ps_sel, \
                 tc.tile_pool(name="ps_g", bufs=1, space="PSUM") as ps_g, \
                 tc.tile_pool(name="ps_big", bufs=2, space="PSUM") as ps_big:

                def l1_unit(t, m, x1t, x2t):
                    if "nol1" in ABLS:
                        return
                    # l1 matmuls + max-pool for one 128-ch chunk of tile t.
                    # Both halves share each lhsT so bass can skip reloads.
                    pls = []
                    for h in range(2):
                        pl = ps_big.tile([128, 512], f32, tag="big",
                                         name=f"pl{h}")
                        nc.tensor.matmul(
                            out=pl[:],
                            lhsT=t_wl1x1[:, m * 128:(m + 1) * 128],
                            rhs=x1t[:, h * 512:(h + 1) * 512],
                            start=True, stop=False)
                        pls.append(pl)
                    for h in range(2):
                        nc.tensor.matmul(
                            out=pls[h][:],
                            lhsT=t_wl1x2[:, m * 128:(m + 1) * 128],
                            rhs=x2t[:, h * 512:(h + 1) * 512],
                            start=False, stop=True)
                    for h in range(2):
                        pv = pls[h][:].rearrange("p (s q) -> p s q",
                                                 q=16)[:, :, 0:15]
                        psl = slice(t * TSTRL + h * 32,
                                    t * TSTRL + (h + 1) * 32)
                        nc.vector.tensor_reduce(out=pooled[m][:, psl],
                                                in_=pv, axis=AX.X, op=OP.max)

                def load_tile(t):
                    c0 = t * TNODES
                    xfc = iop.tile([7, TNODES], f32, tag="xfc")
                    xbc = iop.tile([7, TNODES], f32, tag="xbc")
                    nc.sync.dma_start(out=xfc[:], in_=xefw[:, c0:c0 + TNODES])
                    nc.sync.dma_start(out=xbc[:], in_=xebw[:, c0:c0 + TNODES])
                    return xfc, xbc

                def do_tile(t, prev, cur_io, nxt_io):
                    xfc, xbc = cur_io

                    x1g = xtp.tile([128, TNODES], f32, tag="x1g")
                    x1r2 = xtp.tile([128, TNODES], f32, tag="x1r2")
                    x1t = xtp.tile([65, TNODES], f16, tag="x1t")
                    x2t = xtp.tile([128, TNODES], f16, tag="x2t")
                    nc.vector.memset(x1g[64:128, :], -1.0)
                    nc.vector.memset(x1t[64:65, :], -1.0)

                    for ch in range(2):
                        sl = slice(ch * 512, (ch + 1) * 512)
                        pf = ps_big.tile([128, 512], f32, tag="big")
                        nc.tensor.matmul(out=pf[:], lhsT=t_s1wf[:],
                                         rhs=xfc[:, sl], start=True, stop=True)
                        fwa = s1st.tile([128, 512], f32, tag="fwa")
                        nc.scalar.activation(out=fwa[:], in_=pf[:], func=AF.Relu,
                                             bias=t_s1b[:], scale=t_s1g[:])
                        pb = ps_big.tile([128, 512], f32, tag="big")
                        nc.tensor.matmul(out=pb[:], lhsT=t_s1wb[:],
                                         rhs=xbc[:, sl], start=True, stop=True)
                        bwa = s1st.tile([128, 512], f32, tag="bwa")
                        nc.scalar.activation(out=bwa[:], in_=pb[:], func=AF.Relu,
                                             bias=t_s1b[:], scale=t_s1g[:])
                        xd = s1st.tile([128, 512], f32, tag="xd")
                        nc.vector.tensor_tensor(out=xd[:], in0=fwa[:],
                                                in1=bwa[:], op=OP.add)
                        nc.scalar.copy(out=x1g[0:64, sl], in_=xd[0:64, :])
                        nc.scalar.copy(out=x1t[0:64, sl], in_=xd[0:64, :])
                        nc.scalar.activation(out=x1r2[0:64, sl], in_=xd[0:64, :],
                                             func=AF.Copy, scale=2.0)
                        nc.scalar.activation(out=x1r2[64:128, sl],
                                             in_=xd[64:128, :], func=AF.Square)

                    # ---- kNN + gather + edge MLP, software-pipelined; the
                    # previous tile's l1 units are interleaved to keep every
                    # engine fed while per-block chains cross engines ----
                    SKIP = "noknn" in ABLS
                    Hs, SelTs, Bns, ST = {}, {}, {}, {}
                    for b in range(NBLK + 3 if not SKIP else 0):
                        if b < NBLK:
                            bs = slice(b * 128, (b + 1) * 128)
                            pd = ps_a.tile([128, 128], f32, tag="a")
                            nc.tensor.matmul(out=pd[:], lhsT=x1g[:, bs],
                                             rhs=x1r2[:, bs], start=True,
                                             stop=True)
                            if "nosel" not in ABLS:
                                pdm = knnp.tile([128, 128], f32, tag="pdm")
                                if "nomask" not in ABLS:
                                    nc.vector.tensor_tensor(
                                        out=pdm[:], in0=pd[:],
                                        in1=t_mask[:], op=OP.add)
                                else:
                                    nc.vector.memset(pdm[:], 1.0)
                                m8 = knnp.tile([128, 8], f32, tag="m8")
                                i8 = knnp.tile([128, 8], u16, tag="i8")
                                if "nomax8" not in ABLS:
                                    nc.vector.max(out=m8[:], in_=pdm[:])
                                    nc.vector.max_index(out=i8[:], in_max=m8[:],
                                                        in_values=pdm[:])
                                else:
                                    nc.vector.memset(m8[:], 1.0)
                                    nc.vector.memset(i8[:], 3)
                                i8f = knnp.tile([128, 8], f16, tag="i8f")
                                nc.vector.tensor_copy(out=i8f[:], in_=i8[:])
                                # one-hots for neighbors k=1..4 (k=0 is self,
                                # gathered for free via the identity matrix);
                                # one DVE compare for all four k
                                H = knnp.tile([128, 4 * 128], f16, tag="H")
                                if "noiseq" not in ABLS:
                                    nc.vector.tensor_tensor(
                                        out=H[:].rearrange(
                                            "p (k q) -> p k q", k=4),
                                        in0=t_iota[:].unsqueeze(1)
                                        .to_broadcast([128, 4, 128]),
                                        in1=i8f[:, 1:5].unsqueeze(2)
                                        .to_broadcast([128, 4, 128]),
                                        op=OP.is_equal)
                                else:
                                    nc.vector.memset(H[:], 0.0)
                                Hs[b] = H
                            # B features (node-major) and A term, one
                            # psum tile and one copy for both
                            pba = ps_a.tile([128, 256], f32, tag="a")
                            nc.tensor.matmul(out=pba[:, 0:128],
                                             lhsT=x1t[0:64, bs],
                                             rhs=t_wdt[:], start=True, stop=True)
                            nc.tensor.matmul(out=pba[:, 128:256], lhsT=t_wa[:],
                                             rhs=x1t[:, bs], start=True,
                                             stop=True)
                            BnAn = knnp.tile([128, 256], f16, tag="BnAn")
                            nc.scalar.copy(out=BnAn[:], in_=pba[:])
                            Bns[b] = BnAn
                        if prev is not None and b < 8:
                            l1_unit(t - 1, b, prev["x1t"], prev["x2t"])
                        if 1 <= b < NBLK + 1 and "nosel" not in ABLS:
                            # transpose previous block's one-hots; copy two
                            # blocks' selections per ACT op
                            bb1 = b - 1
                            if bb1 % 2 == 0:
                                ST["selp"] = ps_sel.tile(
                                    [128, 8 * 128], f16, tag="sel",
                                    name="selp")
                                ST["SelT2"] = knnp.tile(
                                    [128, 8 * 128], f16, tag="SelT",
                                    name="SelT2")
                            selp = ST["selp"]
                            half = (bb1 % 2) * 512
                            H = Hs.pop(bb1)
                            for k in range(4):
                                nc.tensor.transpose(
                                    out=selp[:, half + k * 128:
                                             half + (k + 1) * 128],
                                    in_=H[:, k * 128:(k + 1) * 128],
                                    identity=ident16[:])
                            if bb1 % 2 == 1 or bb1 == NBLK - 1:
                                lo = 0
                                hi = half + 512
                                nc.scalar.copy(out=ST["SelT2"][:, lo:hi],
                                               in_=selp[:, lo:hi])
                                SelTs[bb1 - (bb1 % 2)] = ST["SelT2"]
                        if b >= 3 and "nogather" not in ABLS:
                            bb = b - 3
                            bs = slice(bb * 128, (bb + 1) * 128)
                            if "nosel" not in ABLS:
                                SelT2 = SelTs[bb - (bb % 2)]
                                SelT = SelT2[:, (bb % 2) * 512:
                                             (bb % 2) * 512 + 512]
                            else:
                                SelT = None
                            BnAn = Bns.pop(bb)
                            Bn = BnAn[:, 0:128]
                            An = BnAn[:, 128:256]
                            G = ps_g.tile([128, 5 * 128], f32, tag="g")
                            nc.tensor.matmul(out=G[:, 0:512], lhsT=Bn,
                                             rhs=(SelT if SelT is not None
                                                  else t_wl1x2[:, 0:512]),
                                             start=True, stop=True)
                            nc.tensor.matmul(out=G[:, 512:640], lhsT=Bn,
                                             rhs=ident16[:], start=True,
                                             stop=True)
                            x2m = knnp.tile([128, 128], f32, tag="x2m")
                            nc.vector.tensor_reduce(
                                out=x2m[:],
                                in_=G[:].rearrange("c (k p) -> c p k", k=K),
                                axis=AX.X, op=OP.max)
                            x2n = knnp.tile([128, 128], f32, tag="x2n")
                            nc.vector.tensor_tensor(out=x2n[:], in0=x2m[:],
                                                    in1=An, op=OP.add)
                            nc.scalar.activation(out=x2t[:, bs], in_=x2n[:],
                                                 func=AF.Relu)
                    if SKIP:
                        nc.vector.memset(x2t[:], 0.0)
                        if prev is not None:
                            for m in range(8):
                                l1_unit(t - 1, m, prev["x1t"], prev["x2t"])
                    return {"x1t": x1t, "x2t": x2t}

                t1 = [headp.tile([128, BC], f32r, name=f"t1_{o}",
                                 tag=f"t1_{o}") for o in range(4)]
                t2 = [headp.tile([128, BC], f32r, name=f"t2_{o}",
                                 tag=f"t2_{o}") for o in range(2)]
                outs = headp.tile([2, BC], f32, tag="outs")

                def head_half(h):
                    if "nohead" in ABLS:
                        return
                    # ---- head: relu, m1, m2, m3 for one half of the batch;
                    # half 0 only needs tiles 0-7 pooled, so it can run while
                    # tiles 8-15 are still in flight ----
                    osl = slice(h * 512, (h + 1) * 512)
                    h1 = pooled
                    for m in range(8):
                        nc.scalar.activation(out=h1[m][:, osl],
                                             in_=h1[m][:, osl], func=AF.Relu)
                    wm1v = t_wm1[:].rearrange("p (a m) -> p a m", a=8)
                    for o in range(4):
                        pm1 = ps_big.tile([128, 512], f32, tag="big")
                        for kc in range(8):
                            nc.tensor.matmul(
                                out=pm1[:],
                                lhsT=wm1v[:, kc, o * 128:(o + 1) * 128],
                                rhs=h1[kc][:, osl],
                                start=(kc == 0), stop=False)
                        nc.tensor.matmul(
                            out=pm1[:],
                            lhsT=t_bm1[:, o * 128:(o + 1) * 128],
                            rhs=ones_row[:, osl],
                            start=False, stop=True)
                        nc.scalar.activation(out=t1[o][:, osl], in_=pm1[:],
                                             func=AF.Relu)
                    wm2v = t_wm2[:].rearrange("p (a m) -> p a m", a=4)
                    for o in range(2):
                        pm2 = ps_big.tile([128, 512], f32, tag="big")
                        for kc in range(4):
                            nc.tensor.matmul(
                                out=pm2[:],
                                lhsT=wm2v[:, kc, o * 128:(o + 1) * 128],
                                rhs=t1[kc][:, osl],
                                start=(kc == 0), stop=False)
                        nc.tensor.matmul(
                            out=pm2[:],
                            lhsT=t_bm2[:, o * 128:(o + 1) * 128],
                            rhs=ones_row[:, osl],
                            start=False, stop=True)
                        nc.scalar.activation(out=t2[o][:, osl], in_=pm2[:],
                                             func=AF.Relu)
                    wm3v = t_wm3[:].rearrange("p (a m) -> p a m", a=2)
                    pm3 = ps_big.tile([128, 512], f32, tag="big")
                    for kc in range(2):
                        nc.tensor.matmul(
                            out=pm3[0:2, :],
                            lhsT=wm3v[:, kc, :],
                            rhs=t2[kc][:, osl],
                            start=(kc == 0), stop=False)
                    nc.tensor.matmul(out=pm3[0:2, :],
                                     lhsT=t_bm3[:],
                                     rhs=ones_row[:, osl],
                                     start=False, stop=True)
                    nc.scalar.copy(out=outs[:, osl], in_=pm3[0:2, :])

                def whole_body():
                    prev = None
                    cur_io = load_tile(0)
                    for t in range(NTILES):
                        nxt_io = load_tile(t + 1) if t + 1 < NTILES else None
                        prev = do_tile(t, prev, cur_io, nxt_io)
                        cur_io = nxt_io
                        if t == 10:
                            head_half(0)
                    for m in range(8):
                        l1_unit(NTILES - 1, m, prev["x1t"], prev["x2t"])
                    head_half(1)
                    nc.sync.dma_start(out=out_t[:], in_=outs[:])

                REPEAT = int(os.environ.get("KREPEAT", "1"))
                if REPEAT > 1:
                    with tc.For_i(0, REPEAT, 1):
                        whole_body()
                else:
                    whole_body()

    nc.finalize()
    return nc


# ---------------- host-side prep ----------------
def _prep_inputs(pos, edge_index,
                 W_c1fw, b_c1fw, W_c1bw, b_c1bw, g_bn1, be_bn1,
                 W_e, b_e, g_e, be_e,
                 W_l1, b_l1, g_l1, be_l1,
                 W_m1, b_m1, g_m1, be_m1,
                 W_m2, b_m2, g_m2, be_m2,
                 W_m3, b_m3):
    f = np.float32
    h = np.float16
    pos = np.asarray(pos, f)
    E = edge_index.shape[1]
    N = E // 2
    second = np.asarray(edge_index[:, N:])
    first = second[:, ::-1]
    src = np.concatenate([first[0], second[0]])
    dst = np.concatenate([first[1], second[1]])
    xe = np.concatenate([pos[dst] - pos[src], pos[src]], axis=1).astype(f)
    xe = xe.reshape(2 * B_FULL, P, 2 * D)
    fw = xe[:B_FULL]
    bw = xe[B_FULL:][::-1, ::-1, :]

    def pad_t(a):
        # [B, 15, 6] -> per-core feature-major [7, NODES] with ones row
        out = np.zeros((B_FULL, PP, 7), f)
        out[:, :P, :6] = a
        out[:, :, 6] = 1.0
        out = out.reshape(NCORES, NODES, 7)
        return np.ascontiguousarray(out.transpose(0, 2, 1))

    xefw = pad_t(fw)
    xebw = pad_t(bw)

    sq = np.sqrt(np.asarray(1.0 + EPS, f))
    g1 = (np.asarray(g_bn1, f) / sq)
    be1 = np.asarray(be_bn1, f)
    s1g = np.ascontiguousarray(np.tile(g1, 2)[:, None])
    s1b = np.ascontiguousarray(np.tile(be1, 2)[:, None])
    s1wf = np.ascontiguousarray(np.tile(
        np.concatenate([np.asarray(W_c1fw, f),
                        np.asarray(b_c1fw, f)[:, None]], 1).T, (1, 2)))
    s1wb = np.ascontiguousarray(np.tile(
        np.concatenate([np.asarray(W_c1bw, f),
                        np.asarray(b_c1bw, f)[:, None]], 1).T, (1, 2)))

    W_e = np.asarray(W_e, f)
    Wi, Wd = W_e[:, :64], W_e[:, 64:]
    wa = np.ascontiguousarray(
        np.concatenate([(Wi - Wd).T, -np.asarray(b_e, f)[None, :]], 0)).astype(h)
    wdt = np.ascontiguousarray(Wd.T).astype(h)

    ge = np.asarray(g_e, f) / sq
    bee = np.asarray(be_e, f)
    W_l1 = np.asarray(W_l1, f)
    Wl1x1 = W_l1[:, :64]
    Wl1x2 = W_l1[:, 64:] * ge[None, :]
    bl1 = np.asarray(b_l1, f) + W_l1[:, 64:] @ bee
    wl1x1 = np.ascontiguousarray(
        np.concatenate([Wl1x1.T, -bl1[None, :]], 0)).astype(h)
    wl1x2 = np.ascontiguousarray(Wl1x2.T).astype(h)

    def m_fold(W, b, g_prev, be_prev, kchunks):
        # fold previous-layer bn affine into this layer; arrange lhsT
        # [K, M] -> [128, kchunks*M]
        W = np.asarray(W, f)
        gp = np.asarray(g_prev, f) / sq
        Wf = W * gp[None, :]
        bf = np.asarray(b, f) + W @ np.asarray(be_prev, f)
        lhsT = Wf.T  # [K, M]
        Kd, Md = lhsT.shape
        arr = lhsT.reshape(kchunks, 128, Md).transpose(1, 0, 2).reshape(128, -1)
        return np.ascontiguousarray(arr), bf[None, :]

    wm1a, bm1v = m_fold(W_m1, b_m1, g_l1, be_l1, 8)
    wm2a, bm2v = m_fold(W_m2, b_m2, g_m1, be_m1, 4)
    wm3a, bm3v = m_fold(W_m3, b_m3, g_m2, be_m2, 2)

    # additive block-diagonal mask: 0 on own-streamline non-pad columns
    pi = np.arange(128)
    qi = np.arange(128)
    ok = ((pi[:, None] // 16) == (qi[None, :] // 16)) & ((qi[None, :] % 16) != 15)
    maskbd = np.where(ok, 0.0, BIG_NEG).astype(f)
    iotaq = np.tile(np.arange(128, dtype=h)[None, :], (128, 1))

    shared = {
        "s1wf": s1wf, "s1wb": s1wb, "s1g": s1g, "s1b": s1b,
        "wa": wa, "wdt": wdt,
        "wl1x1": wl1x1, "wl1x2": wl1x2,
        "wm1": wm1a, "bm1": bm1v,
        "wm2": wm2a, "bm2": bm2v,
        "wm3": wm3a, "bm3": bm3v,
        "maskbd": maskbd,
        "iotaq": np.ascontiguousarray(iotaq),
        "onesr": np.ones((1, BC), f),
    }
    in_maps = []
    for c in range(NCORES):
        m = dict(shared)
        m["xefw"] = xefw[c]
        m["xebw"] = xebw[c]
        in_maps.append(m)
    return in_maps


def _get_runner():
    if "runner" in _CACHE:
        return _CACHE["runner"]
    from concourse.bass_utils import run_bass_kernel_spmd
    nc = _build_program()
    _CACHE["nc"] = nc

    def runner(in_maps):
        return run_bass_kernel_spmd(nc, in_maps, list(range(NCORES))).results

    _CACHE["runner"] = runner
    return runner


def kernel(**inputs):
    in_maps = _prep_inputs(**inputs)
    results = _get_runner()(in_maps)
    out = np.empty((B_FULL, NCLS), np.float32)
    for c in range(NCORES):
        out[c * BC:(c + 1) * BC, :] = results[c]["out"].T
    return out
